# revision 12
# baseline (speedup 1.0000x reference)
# KL divergence loss kernel for Trainium2 (Bass/Tile), 8-core data-parallel.
#
# Problem: KL(p||q) for diagonal Gaussians over [B=16, L=64, N=512, D=64] f32
# tensors, reduced to a scalar: mean over (B,L) of sum over (N,D) of
#   log(qs/ps) + 0.5*(ps^2 + (pm-qm)^2)/qs^2 - 0.5
#
# Strategy (pure data-parallel, hardcoded):
#   - Shard along B: core c gets B-rows [2c, 2c+2) -> [2,64,512,64], viewed as
#     [128 partitions, 32768 free] (partition = (b,l) pair, free = (n,d)).
#   - The kernel is HBM-bound: per-core DMA engines sustain ~26.8 GB/s x16
#     (428 GB/s) when descriptors never starve; the whole 64 MiB input is a
#     ~158 us transfer. Everything else is structured to (a) keep descriptors
#     flowing with minimal trigger pressure and (b) minimize the serial chain
#     after the LAST byte lands:
#       * HOST-SIDE PACKING: the four tensors are interleaved into ONE DRAM
#         tensor per core, chunk-major: chunk j is a contiguous [128, 4*F]
#         slab ([qm|pm|qs|ps] per partition). One dma_start per chunk moves
#         all four tensors with 128 contiguous 4F*4-byte descriptors (32 KB
#         in steady state) -- 4x fewer ring triggers than per-tensor DMA,
#         which removes the trigger-starvation gaps seen under co-tenant
#         HBM contention.
#       * TAPERED TAIL: ACT is the tail-rate limiter (Ln(F) + Square(2F) =
#         ~2.7 ns/elem vs 4.79 ns/elem transfer). Chunk sizes taper as
#         [1536, 1152, 896, 704, 576, 480, 416, 384] chosen so each chunk's
#         ACT work fits inside the next chunk's transfer window; the post-
#         last-byte chain is only the final 384-elem chunk's work.
#   - Math avoids division (ACT Reciprocal is blocked):
#       w  = reciprocal_approx_fast(qs) = 1/qs   (DVE custom op, ~51 ULP)
#       r1 = ps * w, r2 = (pm-qm) * w  (DVE TT into one wide tile [r2 | r1]
#         so a SINGLE ACT Square+accum yields S2+S3 = S23)
#       S1 += sum ln(r1)   (Ln and Square share one ACT table set; accum_out
#         gives free reduces; full-size ACT outs parked in PSUM scratch)
#       d = pm-qm runs on the otherwise-idle GpSimd engine.
#   - Each core DMAs out its [128, 2*NCH] partial-sum accumulators; the host
#     combines in float64:  mean = (-S1 + 0.5*S23)/(B*L) - N*D/2.

import numpy as np

B, L, N, D = 16, 64, 512, 64
NCORES = 8
P = 128                      # SBUF partitions = per-core B*L = (B/NCORES)*L
TOT = N * D                  # free elements per partition = 32768
# Taper solved against per-engine rates so no engine accumulates backlog:
#   Vector (1.04 ns/elem x3 ops), ACT (0.9 ns/elem x3F), GpSimd (2.05F+900):
#   c(F_{j-1}) + 2c(F_j) <= transfer(F_{j+1}) = 4.79F_{j+1}ns, etc.
# First chunk small so the HWDGE generates its descriptors quickly and the
# DMA engines spin up ~1-2us earlier.
CHUNKS = [512, 1536] + [2048] * 11 + [1792, 1472, 1184, 928, 768, 640, 544, 480, 384]
NCH = len(CHUNKS)
assert sum(CHUNKS) == TOT
FMAX = max(CHUNKS)

_CACHE = {}


def build_nc():
    from contextlib import ExitStack
    import concourse.tile as tile
    from concourse import bacc, mybir

    dt = mybir.dt.float32
    AF = mybir.ActivationFunctionType

    nc = bacc.Bacc(
        "TRN2", target_bir_lowering=False, debug=False, num_devices=NCORES
    )
    inp = nc.dram_tensor("inp", [P, 4 * TOT], dt, kind="ExternalInput").ap()
    out = nc.dram_tensor("acc_out", [P, 2 * NCH], dt, kind="ExternalOutput").ap()

    with tile.TileContext(nc) as tc, ExitStack() as ctx:
        io = ctx.enter_context(tc.tile_pool(name="io", bufs=4))
        rp = ctx.enter_context(tc.tile_pool(name="rp", bufs=3))
        accp = ctx.enter_context(tc.tile_pool(name="accp", bufs=1))
        acc = accp.tile([P, 2 * NCH], dt)
        # ACT needs a full-size out even when only accum_out matters; park it
        # in one PSUM scratch tile (PSUM is otherwise unused — no matmuls —
        # and ScalarE->PSUM is the faster port; WAW on ACT only, serial there).
        psp = ctx.enter_context(tc.tile_pool(name="psp", bufs=1, space="PSUM"))
        scr = psp.tile([P, 2 * FMAX], dt)

        o4 = 0
        prev = None  # (r_t, j, F) of previous chunk
        for j, F in enumerate(CHUNKS):
            io_t = io.tile([P, 4 * F], dt, tag="io_t")
            nc.sync.dma_start(io_t[:], inp[:, o4:o4 + 4 * F])
            o4 += 4 * F
            qm = io_t[:, 0:F]
            pm = io_t[:, F:2 * F]
            qs = io_t[:, 2 * F:3 * F]
            ps = io_t[:, 3 * F:4 * F]

            # Result tile [d | r1 | w]: ALL io_t readers (sub, recip,
            # mul_r1) fire right when chunk j lands, so the io slot frees
            # ~2 vector-ops after landing — the DMA WAR gate never waits on
            # the (later) r2/Square flush chain. [d | r1] are adjacent so
            # ONE wide ACT Square+accum still yields S2+S3 in one op.
            r_t = rp.tile([P, 3 * F], dt, tag="r_t")

            # PREVIOUS chunk's r2 chain (one-chunk software pipeline): its
            # inputs are long since ready, so the in-order DVE/ACT queues
            # never block here; at the tail it drains chunk j-1 entirely
            # during chunk j's transfer.
            if prev is not None:
                r_p, jp, Fp = prev
                # r2 = d*w, in place over the d slot
                nc.vector.tensor_mul(
                    r_p[:, :Fp], r_p[:, :Fp], r_p[:, 2 * Fp:3 * Fp]
                )
                nc.scalar.activation(
                    scr[:, :2 * Fp], r_p[:, :2 * Fp], AF.Square,
                    accum_out=acc[:, NCH + jp:NCH + jp + 1],
                )

            # d = pm - qm on the otherwise-idle GpSimd engine — except for
            # the LAST chunk, where GpSimd's ~1us op + cross-engine sem hop
            # would sit on the post-last-byte critical path; DVE does it.
            if j < NCH - 1:
                nc.gpsimd.tensor_sub(r_t[:, 0:F], pm, qm)
            else:
                nc.vector.tensor_sub(r_t[:, 0:F], pm, qm)
            # w = 1/qs (single custom-DVE op, ~51 ULP)
            nc.vector.reciprocal_approx_fast(out=r_t[:, 2 * F:3 * F], in_=qs)
            nc.vector.tensor_mul(r_t[:, F:2 * F], ps, r_t[:, 2 * F:3 * F])
            # S1 += sum ln r1
            # (Ln and Square share one ACT table set -> single table load)
            nc.scalar.activation(
                scr[:, :F], r_t[:, F:2 * F], AF.Ln, accum_out=acc[:, j:j + 1]
            )
            prev = (r_t, j, F)

        # last chunk's r2 chain (short: F=384, ~0.7us wide-Square)
        r_p, jp, Fp = prev
        nc.vector.tensor_mul(r_p[:, :Fp], r_p[:, :Fp], r_p[:, 2 * Fp:3 * Fp])
        nc.scalar.activation(
            scr[:, :2 * Fp], r_p[:, :2 * Fp], AF.Square,
            accum_out=acc[:, NCH + jp:NCH + jp + 1],
        )
        nc.scalar.dma_start(out[:], acc[:])

    nc.compile()
    return nc


def _shard(a, c):
    a = np.asarray(a, dtype=np.float32)
    return np.ascontiguousarray(a[2 * c:2 * c + 2]).reshape(P, TOT)


def make_in_maps(prior_mu, prior_sigma, post_mu, post_sigma):
    maps = []
    for c in range(NCORES):
        qm = _shard(post_mu, c)
        pm = _shard(prior_mu, c)
        qs = _shard(post_sigma, c)
        ps = _shard(prior_sigma, c)
        X = np.empty((P, 4 * TOT), dtype=np.float32)
        off = o4 = 0
        for F in CHUNKS:
            sl = np.s_[:, off:off + F]
            X[:, o4:o4 + F] = qm[sl]
            X[:, o4 + F:o4 + 2 * F] = pm[sl]
            X[:, o4 + 2 * F:o4 + 3 * F] = qs[sl]
            X[:, o4 + 3 * F:o4 + 4 * F] = ps[sl]
            off += F
            o4 += 4 * F
        maps.append({"inp": X})
    return maps


def combine(results):
    S1 = S23 = 0.0
    for r in results:
        a = r["acc_out"].astype(np.float64)
        S1 += a[:, :NCH].sum()
        S23 += a[:, NCH:].sum()
    mean = (-S1 + 0.5 * S23) / (B * L) - 0.5 * N * D
    return np.float32(mean)


def _build_exec():
    """Compile the NEFF and build a jitted shard_map executor (mirrors
    bass2jax.run_bass_via_pjrt's multi-core path). Returning it separately
    lets kernel() pre-place inputs on device and BLOCK before launching, so
    the 64 MiB/core input uploads never overlap kernel execution (PJRT
    otherwise streams parameter uploads concurrently with the first cores'
    execution, stealing DMA-queue bandwidth from the kernel)."""
    import jax
    import numpy as np2
    from jax.sharding import Mesh, PartitionSpec, NamedSharding
    from jax.experimental.shard_map import shard_map
    from concourse import mybir
    from concourse.bass2jax import (
        _bass_exec_p, partition_id_tensor, install_neuronx_cc_hook,
    )

    nc = build_nc()
    install_neuronx_cc_hook()
    partition_name = nc.partition_id_tensor.name if nc.partition_id_tensor else None
    in_names, out_names, out_avals = [], [], []
    for alloc in nc.m.functions[0].allocations:
        if not isinstance(alloc, mybir.MemoryLocationSet):
            continue
        name = alloc.memorylocations[0].name
        if alloc.kind == "ExternalInput":
            if name != partition_name:
                in_names.append(name)
        elif alloc.kind == "ExternalOutput":
            out_names.append(name)
            out_avals.append(jax.core.ShapedArray(
                tuple(alloc.tensor_shape), mybir.dt.np(alloc.dtype)))
    n_params = len(in_names)
    n_outs = len(out_avals)
    all_in = in_names + out_names + ([partition_name] if partition_name else [])

    def _body(*args):
        operands = list(args)
        if partition_name is not None:
            operands.append(partition_id_tensor())
        return tuple(_bass_exec_p.bind(
            *operands,
            out_avals=tuple(out_avals), in_names=tuple(all_in),
            out_names=tuple(out_names), lowering_input_output_aliases=(),
            sim_require_finite=True, sim_require_nnan=True, nc=nc,
        ))

    devices = jax.devices()[:NCORES]
    mesh = Mesh(np2.asarray(devices), ("core",))
    spec = NamedSharding(mesh, PartitionSpec("core"))
    sharded = jax.jit(
        shard_map(_body, mesh=mesh,
                  in_specs=(PartitionSpec("core"),) * (n_params + n_outs),
                  out_specs=(PartitionSpec("core"),) * n_outs, check_rep=False),
        donate_argnums=tuple(range(n_params, n_params + n_outs)),
        keep_unused=True,
    )
    return {
        "sharded": sharded, "spec": spec, "in_names": in_names,
        "out_names": out_names, "out_avals": out_avals, "nc": nc,
    }


def kernel(prior_mu, prior_sigma, post_mu, post_sigma):
    import jax

    if "exec" not in _CACHE:
        _CACHE["exec"] = _build_exec()
    ex = _CACHE["exec"]
    in_maps = make_in_maps(prior_mu, prior_sigma, post_mu, post_sigma)
    concat_in = [
        np.concatenate([in_maps[c][nm] for c in range(NCORES)], axis=0)
        for nm in ex["in_names"]
    ]
    concat_zeros = [
        np.zeros((NCORES * a.shape[0], *a.shape[1:]), a.dtype)
        for a in ex["out_avals"]
    ]
    # Pre-place and BLOCK so input-upload DMA finishes before exec starts.
    dev_in = [jax.device_put(a, ex["spec"]) for a in concat_in]
    dev_zero = [jax.device_put(a, ex["spec"]) for a in concat_zeros]
    jax.block_until_ready(dev_in)
    jax.block_until_ready(dev_zero)
    out_arrs = ex["sharded"](*dev_in, *dev_zero)
    results = [
        {nm: np.asarray(out_arrs[i]).reshape(NCORES, *ex["out_avals"][i].shape)[c]
         for i, nm in enumerate(ex["out_names"])}
        for c in range(NCORES)
    ]
    return combine(results)
